# revision 1
# baseline (speedup 1.0000x reference)
"""RNN-T decoder (embedding + 2-layer LSTM + joint network) on 8 Trainium2 cores.

Strategy:
  - LSTM (B=4, U=64, D=1024) is inherently sequential with a tiny batch; it runs
    redundantly on all 8 cores near its PE floor. Input projections are batched
    over all steps; the per-step recurrent matmul streams W_hh through the PE
    with h^T as the (tiny) stationary operand.
  - Joint network (dominant FLOPs) is sharded over T: core c computes
    out[:, c*16:(c+1)*16, :, :].
  - All weights are pre-transposed on the host so DMA loads are contiguous.

kernel(**inputs) takes the full unsharded inputs (as in reference.setup_inputs)
and returns the full (B, T, U, ODIM) float32 output.
"""
import sys
import numpy as np

sys.path.insert(0, "/opt/trn_rl_repo")

import concourse.bass as bass
import concourse.bacc as bacc
import concourse.mybir as mybir
import concourse.tile as tile
from concourse.bass_utils import run_bass_kernel_spmd
from concourse.masks import make_identity
from contextlib import ExitStack

F32 = mybir.dt.float32
F32R = mybir.dt.float32r
BF16 = mybir.dt.bfloat16
I32 = mybir.dt.int32
AF = mybir.ActivationFunctionType
OP = mybir.AluOpType

B, T, U = 4, 128, 64
NCORES = 8
TC = T // NCORES          # 16 T-columns per core
E, D, G = 512, 1024, 4096  # embed, dunits, 4*dunits
J, O = 512, 2048           # joint dim, odim
UB = U * B                 # 256, u-major token index
BT = B * TC                # 64 encoder rows per core
NB = G // 512              # 8 gate blocks of 512
KD = D // 128              # 8 contraction chunks of hidden dim
# gate block order for streaming: g first, then f, i, o (c-chain starts early,
# o last since its only consumer is the final h multiply)
NBORDER = [4, 5, 2, 3, 0, 1, 6, 7]

_CACHE = {}


class _SkipJoint(Exception):
    pass


def _mm_r(nc, out, lhsT, rhs, **kw):
    """matmul with both operands viewed as float32r (full-rate fp32 storage)."""
    nc.tensor.matmul(out, lhsT=lhsT.bitcast(F32R), rhs=rhs.bitcast(F32R), **kw)


def _emit_xproj(nc, pools, rhs_of_ec, w_of_ec, nchunks,
                bih, bhh, gxT, accum, with_bias):
    """gxT[gm][:, :] (+)= (W @ x^T) block + bias, for 32 g-tiles of 128.

    rhs_of_ec: ec -> [128, 256] rhs tile (x^T chunk, K on partitions)
    w_of_ec:   ec -> [128, G] tile of W.T rows for that chunk (lhsT source)
    bih/bhh: DRAM bias handles (summed on device), used when with_bias
    accum: False -> overwrite gxT, True -> add into gxT
    """
    pbig, bpool = pools["pbig"], pools["bias"]
    ones_r = pools["ones"]
    for gm in range(32):
        gs = slice(128 * (gm % 4), 128 * (gm % 4) + 128)
        if with_bias and gm % 4 == 0:
            cb = slice((gm // 4) * 512, (gm // 4) * 512 + 512)
            ba = bpool.tile([1, 512], F32R, tag="ba", name="ba")
            bb = bpool.tile([1, 512], F32, tag="bb", name="bb")
            nc.sync.dma_start(ba[:1, :], bih[None, cb].bitcast(F32R))
            nc.sync.dma_start(bb[:1, :], bhh[None, cb])
            nc.vector.tensor_tensor(ba[:1, :], in0=ba[:1, :],
                                    in1=bb[:1, :], op=OP.add)
        ps = pbig.tile([128, 256], F32, tag="pbig", name="pbig")
        for ec in range(nchunks):
            _mm_r(nc, ps[:],
                  lhsT=w_of_ec(ec)[:, 128 * gm: 128 * (gm + 1)],
                  rhs=rhs_of_ec(ec),
                  start=(ec == 0), stop=(ec == nchunks - 1 and not with_bias))
        if with_bias:
            _mm_r(nc, ps[:], lhsT=ba[:1, gs], rhs=ones_r[:1, :256],
                  start=False, stop=True)
        if accum:
            nc.vector.tensor_tensor(gxT[gm][:], in0=gxT[gm][:], in1=ps[:],
                                    op=OP.add)
        else:
            nc.vector.tensor_copy(gxT[gm][:], ps[:])


def _emit_lstm_layer(nc, pools, ident, whh, gxT, hT_all, c_sb, gates, hbuf,
                     steps):
    """One LSTM layer, `steps` sequential steps.

    whh: 8 SBUF tiles [128, G] = W_hh.T chunks (rhs stream)
    gxT: 32 SBUF tiles [128, 256] = batched (W_ih x + bias)^T, cols ub = 4u+b
    hT_all: 8 SBUF tiles [128, 4*(U+1)]; col block u = h[u-1].T chunk
            (block 0 = zeros); this layer writes block u+1.
    """
    pgates, ptr = pools["pgates"], pools["ptr"]
    nborder = pools.get("nborder") or NBORDER
    ident4 = ident[:4, :4]
    for u in range(steps):
        stat = [hT_all[k][:, 4 * u: 4 * u + 4] for k in range(KD)]
        us = slice(4 * u, 4 * u + 4)
        for nb in nborder:
            nbs = slice(nb * 512, (nb + 1) * 512)
            ps = pgates.tile([4, 512], F32, tag="pg", name="pg")
            # inject the batched x-projection first (ps[:, 128c:] = gxT[...].T):
            # it depends only on gxT, so the PE can start it before this
            # step's h^T copies land
            identb = pools["identb"]
            if pools.get("inject_first", True):
                for c in range(4):
                    nc.tensor.matmul(ps[:, 128 * c:128 * (c + 1)],
                                     lhsT=gxT[4 * nb + c][:, us], rhs=identb[:],
                                     start=(c == 0), stop=False)
                for k in range(KD):
                    _mm_r(nc, ps[:], lhsT=stat[k], rhs=whh[k][:, nbs],
                          start=False, stop=(k == KD - 1))
            else:
                for k in range(KD):
                    _mm_r(nc, ps[:], lhsT=stat[k], rhs=whh[k][:, nbs],
                          start=(k == 0), stop=False)
                for c in range(4):
                    nc.tensor.matmul(ps[:, 128 * c:128 * (c + 1)],
                                     lhsT=gxT[4 * nb + c][:, us], rhs=identb[:],
                                     start=False, stop=(c == 3))
            if pools.get("tanh_only") and nb not in (4, 5):
                # sigmoid(x) = 0.5*tanh(x/2) + 0.5 — keeps ACT on one table set
                nc.scalar.activation(gates[:, nbs], ps[:], AF.Tanh, scale=0.5)
                nc.vector.tensor_scalar(gates[:, nbs], gates[:, nbs], 0.5, 0.5,
                                        OP.mult, OP.add)
            else:
                fn = pools.get("act_fn") or (AF.Tanh if nb in (4, 5) else AF.Sigmoid)
                nc.scalar.activation(gates[:, nbs], ps[:], fn)
        # c' = sig(f)*c + sig(i)*tanh(g);  h = sig(o)*tanh(c')
        # transposes of each 512-half issue as soon as that half of h is ready
        for hh in range(2):
            s = slice(hh * 512, (hh + 1) * 512)
            gi = gates[:, 0 * D:][:, s]
            gf = gates[:, 1 * D:][:, s]
            gg = gates[:, 2 * D:][:, s]
            go = gates[:, 3 * D:][:, s]
            ch = c_sb[:, s]
            nc.vector.tensor_tensor(gf, in0=gf, in1=ch, op=OP.mult)      # f*c
            nc.vector.tensor_tensor(gi, in0=gi, in1=gg, op=OP.mult)      # i*g
            nc.vector.tensor_tensor(ch, in0=gi, in1=gf, op=OP.add)       # c'
            nc.scalar.activation(gg, ch, pools.get("act_fn") or AF.Tanh)
            nc.vector.tensor_tensor(hbuf[:, s], in0=go, in1=gg, op=OP.mult)
            for k in range(4 * hh, 4 * hh + 4):
                tp = ptr.tile([128, 4], F32, tag="tr", name="tr")
                nc.tensor.transpose(tp[:], in_=hbuf[:, 128 * k: 128 * (k + 1)],
                                    identity=ident4)
                eng = pools.get("ht_engine", "vector")
                getattr(nc, eng).tensor_copy(
                    hT_all[k][:, 4 * (u + 1): 4 * (u + 1) + 4], tp[:])


def build_nc(steps=U, layers=2, joint=True, act_fn=None, tanh_only=False,
             pg_bufs=3, ptr_bufs=2, ht_engine='vector', inject_first=True,
             nborder=None):
    nc = bacc.Bacc("TRN2", target_bir_lowering=False, debug=False)

    hs = nc.dram_tensor("hs", [BT, E], F32, kind="ExternalInput")
    ys_idx = nc.dram_tensor("ys_idx", [UB], I32, kind="ExternalInput")
    embed = nc.dram_tensor("embed", [O, E], F32, kind="ExternalInput")
    wih0T = nc.dram_tensor("wih0T", [E, G], F32R, kind="ExternalInput")
    whh0T = nc.dram_tensor("whh0T", [D, G], F32R, kind="ExternalInput")
    wih1T = nc.dram_tensor("wih1T", [D, G], F32R, kind="ExternalInput")
    whh1T = nc.dram_tensor("whh1T", [D, G], F32R, kind="ExternalInput")
    bih0 = nc.dram_tensor("bih0", [G], F32, kind="ExternalInput")
    bhh0 = nc.dram_tensor("bhh0", [G], F32, kind="ExternalInput")
    bih1 = nc.dram_tensor("bih1", [G], F32, kind="ExternalInput")
    bhh1 = nc.dram_tensor("bhh1", [G], F32, kind="ExternalInput")
    wencT = nc.dram_tensor("wencT", [E, J], F32R, kind="ExternalInput")
    wdecT = nc.dram_tensor("wdecT", [D, J], F32R, kind="ExternalInput")
    woutT = nc.dram_tensor("woutT", [J, O], F32R, kind="ExternalInput")
    benc = nc.dram_tensor("benc", [J], F32R, kind="ExternalInput")
    bout_bc = nc.dram_tensor("bout_bc", [128, O], F32, kind="ExternalInput")
    ones_d = nc.dram_tensor("ones_d", [256], F32R, kind="ExternalInput")
    ident_f = nc.dram_tensor("ident_f", [128, 128], F32, kind="ExternalInput")
    ident_b = nc.dram_tensor("ident_b", [128, 128], BF16, kind="ExternalInput")
    out = nc.dram_tensor("out", [BT * U, O], F32, kind="ExternalOutput")

    with tile.TileContext(nc) as tc, ExitStack() as es:
        cpool = es.enter_context(tc.tile_pool(name="const", bufs=1))
        ppool = es.enter_context(tc.tile_pool(name="persist", bufs=1))

        ident = cpool.tile([128, 128], F32, tag="ident", name="ident")
        nc.sync.dma_start(ident[:], ident_f[:])
        ones_r = cpool.tile([1, 256], F32R, tag="ones", name="ones")
        nc.sync.dma_start(ones_r[:1, :], ones_d[None, :])
        identb = cpool.tile([128, 128], BF16, tag="identb", name="identb")
        nc.sync.dma_start(identb[:], ident_b[:])

        gxT = [ppool.tile([128, 256], BF16, tag=f"gxT{g}", name=f"gxT{g}")
               for g in range(32)]
        hT0 = [ppool.tile([128, 4 * (U + 1)], F32R, tag=f"hT0_{k}", name=f"hT0_{k}")
               for k in range(KD)]
        hT1 = [ppool.tile([128, 4 * (U + 1)], F32R, tag=f"hT1_{k}", name=f"hT1_{k}")
               for k in range(KD)]
        gates = ppool.tile([4, G], F32, tag="gates", name="gates")
        c_sb = ppool.tile([4, D], F32, tag="c", name="c")
        hbuf = ppool.tile([4, D], F32, tag="h", name="h")
        for k in range(KD):
            nc.gpsimd.memset(hT0[k][:].bitcast(F32), 0.0)
            nc.gpsimd.memset(hT1[k][:].bitcast(F32), 0.0)
        nc.gpsimd.memset(c_sb[:], 0.0)

        pools = {"ones": ones_r, "act_fn": act_fn, "identb": identb,
                 "tanh_only": tanh_only, "ht_engine": ht_engine,
                 "inject_first": inject_first, "nborder": nborder}

        wenc = [ppool.tile([128, J], F32R, tag=f"wenc{ec}", name=f"wenc{ec}")
                for ec in range(4)]
        benc_sb = ppool.tile([1, J], F32R, tag="benc", name="benc")
        hs_sb = ppool.tile([BT, E], F32, tag="hs_sb", name="hs_sb")
        hsT = [ppool.tile([128, BT], F32R, tag=f"hsT{ec}", name=f"hsT{ec}")
               for ec in range(4)]
        encp = [ppool.tile([128, BT], F32, tag=f"encp{jt}", name=f"encp{jt}")
                for jt in range(4)]

        # ---- Phase 1+2: embedding gather -> eys^T, layer-0 x-projection ----
        with tc.tile_pool(name="ph2", bufs=1) as p2, \
             tc.tile_pool(name="bias2", bufs=2) as bpool2, \
             tc.tile_pool(name="pbig", bufs=4, space="PSUM") as pbig, \
             tc.tile_pool(name="pT", bufs=2, space="PSUM") as pT:
            pools["pbig"] = pbig
            pools["bias"] = bpool2
            # issue the tiny idx DMAs before the 8MB W_ih0 load: they gate
            # the (SWDGE) embedding gather and the first PE transposes
            eysT = [p2.tile([128, 256], F32R, tag=f"eysT{ec}", name=f"eysT{ec}") for ec in range(4)]
            idxs = []
            for t in range(2):
                idx = p2.tile([128, 1], I32, tag=f"idx{t}", name=f"idx{t}")
                nc.sync.dma_start(idx[:, :1], ys_idx[128 * t:128 * (t + 1), None])
                idxs.append(idx)
            wih0 = [p2.tile([128, G], F32R, tag=f"wih0_{ec}", name=f"wih0_{ec}") for ec in range(4)]
            for ec in range(4):
                nc.sync.dma_start(wih0[ec][:], wih0T[128 * ec:128 * (ec + 1), :])
            for t in range(2):
                idx = idxs[t]
                ey = p2.tile([128, E], F32, tag=f"ey{t}", name=f"ey{t}")
                nc.gpsimd.indirect_dma_start(
                    out=ey[:], out_offset=None, in_=embed[:],
                    in_offset=bass.IndirectOffsetOnAxis(ap=idx[:, :1], axis=0))
                for ec in range(4):
                    tp = pT.tile([128, 128], F32, tag="pT", name="pT")
                    nc.tensor.transpose(tp[:], in_=ey[:, 128 * ec:128 * (ec + 1)],
                                        identity=ident[:])
                    nc.vector.tensor_copy(eysT[ec][:, 128 * t:128 * (t + 1)], tp[:])
            _emit_xproj(nc, pools, lambda ec: eysT[ec][:], lambda ec: wih0[ec],
                        4, bih0, bhh0, gxT, accum=False, with_bias=True)

            # encoder-side joint work: hs^T and enc_p^T (PE has slack here)
            for ec in range(4):
                nc.sync.dma_start(wenc[ec][:], wencT[128 * ec:128 * (ec + 1), :])
            nc.sync.dma_start(hs_sb[:], hs[:])
            nc.sync.dma_start(benc_sb[:1, :], benc[None, :])
            for ec in range(4):
                tp = pT.tile([128, 128], F32, tag="pT", name="pT")
                nc.tensor.transpose(tp[:, :BT],
                                    in_=hs_sb[:, 128 * ec:128 * (ec + 1)],
                                    identity=ident[:BT, :BT])
                nc.vector.tensor_copy(hsT[ec][:], tp[:, :BT])
            for jt in range(4):
                tpp = pT.tile([128, 128], F32, tag="pT", name="pT")
                pse = tpp[:, :BT]
                for ec in range(4):
                    _mm_r(nc, pse, lhsT=wenc[ec][:, 128 * jt:128 * (jt + 1)],
                          rhs=hsT[ec][:], start=(ec == 0), stop=False)
                _mm_r(nc, pse, lhsT=benc_sb[:1, 128 * jt:128 * (jt + 1)],
                      rhs=ones_r[:1, :BT], start=False, stop=True)
                nc.vector.tensor_copy(encp[jt][:], pse)

        # ---- Phase 3: layer-0 recurrence ----
        with tc.tile_pool(name="whhA", bufs=1) as whhp, \
             tc.tile_pool(name="pgatesA", bufs=pg_bufs, space="PSUM") as pgates, \
             tc.tile_pool(name="ptrA", bufs=ptr_bufs, space="PSUM") as ptr:
            pools["pgates"], pools["ptr"] = pgates, ptr
            whh = [whhp.tile([128, G], F32R, tag=f"whh{k}", name=f"whh{k}") for k in range(KD)]
            for k in range(KD):
                nc.sync.dma_start(whh[k][:], whh0T[128 * k:128 * (k + 1), :])
            _emit_lstm_layer(nc, pools, ident[:], whh, gxT, hT0, c_sb,
                             gates, hbuf, steps)

        # ---- Phase 4: layer-1 x-projection (streams W_ih1, accumulates) ----
        with tc.tile_pool(name="ph4", bufs=1) as p4, \
             tc.tile_pool(name="bias4", bufs=2) as bpool4, \
             tc.tile_pool(name="pbig2", bufs=4, space="PSUM") as pbig2:
            pools["pbig"] = pbig2
            pools["bias"] = bpool4
            wbuf = [p4.tile([128, G], F32R, tag=f"wih1_{i}", name=f"wih1_{i}") for i in range(4)]
            for p in range(4):
                for i in range(2):
                    k = 2 * p + i
                    nc.sync.dma_start(wbuf[(2 * p + i) % 4][:],
                                      wih1T[128 * k:128 * (k + 1), :])
                _emit_xproj(nc, pools,
                        lambda i, p=p: hT0[2 * p + i][:, 4:4 + 4 * U],
                        lambda i, p=p: wbuf[(2 * p + i) % 4], 2, bih1, bhh1, gxT,
                        accum=(p > 0), with_bias=(p == 3))

        # ---- Phase 5: layer-1 recurrence ----
        nc.gpsimd.memset(c_sb[:], 0.0)
        if layers < 2:
            for k in range(KD):
                nc.vector.tensor_copy(hT1[k][:, 4:4 + 4 * steps],
                                      hT0[k][:, 4:4 + 4 * steps])
        with tc.tile_pool(name="whhB", bufs=1) as whhp2, \
             tc.tile_pool(name="pgatesB", bufs=pg_bufs, space="PSUM") as pgates2, \
             tc.tile_pool(name="ptrB", bufs=ptr_bufs, space="PSUM") as ptr2:
            pools["pgates"], pools["ptr"] = pgates2, ptr2
            if layers >= 2:
                whh2 = [whhp2.tile([128, G], F32R, tag=f"whh2_{k}", name=f"whh2_{k}") for k in range(KD)]
                for k in range(KD):
                    nc.sync.dma_start(whh2[k][:], whh1T[128 * k:128 * (k + 1), :])
                _emit_lstm_layer(nc, pools, ident[:], whh2, gxT, hT1, c_sb,
                                 gates, hbuf, steps)

        # ---- Phase 6: joint network on this core's T-slice ----
        if not joint:
            # still must write the output: cheap memset-like DMA from gates
            zsrc = ppool.tile([128, 512], F32, tag="zsrc", name="zsrc")
            nc.gpsimd.memset(zsrc[:], 0.0)
            for m0 in range(BT * U // 128):
                for ob in range(4):
                    nc.sync.dma_start(out[128 * m0:128 * (m0 + 1),
                                          ob * 512:(ob + 1) * 512], zsrc[:])
        import contextlib
        with contextlib.suppress(_SkipJoint), \
             tc.tile_pool(name="joint", bufs=1) as jp, \
             tc.tile_pool(name="zt", bufs=4) as ztp, \
             tc.tile_pool(name="osb", bufs=4) as osbp, \
             tc.tile_pool(name="pj", bufs=4, space="PSUM") as pj, \
             tc.tile_pool(name="pT2", bufs=2, space="PSUM") as pT2:
            if not joint:
                raise _SkipJoint
            wdec = [jp.tile([128, J], F32R, tag=f"wdec{k}", name=f"wdec{k}") for k in range(KD)]
            wout = [jp.tile([128, O], F32R, tag=f"wout{jt}", name=f"wout{jt}") for jt in range(4)]
            bout_sb = jp.tile([128, O], F32, tag="bout", name="bout")
            decp = [jp.tile([128, 256], F32, tag=f"decp{jt}", name=f"decp{jt}") for jt in range(4)]
            for k in range(KD):
                nc.sync.dma_start(wdec[k][:], wdecT[128 * k:128 * (k + 1), :])
            nc.sync.dma_start(bout_sb[:], bout_bc[:])
            for jt in range(4):
                nc.sync.dma_start(wout[jt][:], woutT[128 * jt:128 * (jt + 1), :])

            # dec_p^T[jt], columns reordered (b, u)
            for jt in range(4):
                ps = pj.tile([128, 256], F32, tag="pj", name="pj")
                for k in range(KD):
                    rhs = hT1[k][:, 4:4 + 4 * U].rearrange("p (u b) -> p b u",
                                                           u=U, b=B)
                    _mm_r(nc, ps[:], lhsT=wdec[k][:, 128 * jt:128 * (jt + 1)],
                          rhs=rhs, start=(k == 0), stop=(k == KD - 1))
                nc.vector.tensor_copy(decp[jt][:], ps[:])
            # z^T tiles + output matmul, one M-tile (=2 encoder rows) at a time
            for m in range(BT * U // 128):
                zt = [ztp.tile([128, 128], F32R, tag=f"zt{jt}", name=f"zt{jt}") for jt in range(4)]
                for jt in range(4):
                    for half in range(2):
                        bt = 2 * m + half
                        b = bt // TC
                        nc.scalar.activation(
                            zt[jt][:, half * 64:(half + 1) * 64],
                            decp[jt][:, b * 64:(b + 1) * 64],
                            AF.Tanh, bias=encp[jt][:, bt:bt + 1])
                for ob in range(4):
                    obs = slice(ob * 512, (ob + 1) * 512)
                    ps = pj.tile([128, 512], F32, tag="pj", name="pj")
                    for jt in range(4):
                        _mm_r(nc, ps[:], lhsT=zt[jt][:], rhs=wout[jt][:, obs],
                              start=(jt == 0), stop=(jt == 3))
                    o_sb = osbp.tile([128, 512], F32, tag="osb", name="osb")
                    nc.vector.tensor_tensor(o_sb[:], in0=ps[:], in1=bout_sb[:, obs],
                                            op=OP.add)
                    nc.sync.dma_start(out[128 * m:128 * (m + 1), obs], o_sb[:])

    nc.compile()
    return nc


def _prep_inputs(hs_pad, ys_in_pad, embed, W_ih0, W_hh0, b_ih0, b_hh0,
                 W_ih1, W_hh1, b_ih1, b_hh1, W_enc, b_enc, W_dec, W_out, b_out):
    f = np.float32
    tr = lambda a: np.ascontiguousarray(np.asarray(a).T, dtype=f)
    common = {
        "ys_idx": np.ascontiguousarray(np.asarray(ys_in_pad).T.reshape(-1),
                                       dtype=np.int32),
        "embed": np.ascontiguousarray(embed, dtype=f),
        "wih0T": tr(W_ih0), "whh0T": tr(W_hh0),
        "wih1T": tr(W_ih1), "whh1T": tr(W_hh1),
        "bih0": np.asarray(b_ih0, f), "bhh0": np.asarray(b_hh0, f),
        "bih1": np.asarray(b_ih1, f), "bhh1": np.asarray(b_hh1, f),
        "wencT": tr(W_enc), "wdecT": tr(W_dec), "woutT": tr(W_out),
        "benc": np.asarray(b_enc, f),
        "bout_bc": np.ascontiguousarray(
            np.broadcast_to(np.asarray(b_out, f)[None, :], (128, O))),
        "ones_d": np.ones(256, f),
        "ident_f": np.eye(128, dtype=f),
        "ident_b": np.eye(128).astype(np.dtype("bfloat16") if hasattr(np, "bfloat16")
                                      else __import__("ml_dtypes").bfloat16),
    }
    hs_np = np.asarray(hs_pad, f)
    in_maps = []
    for c in range(NCORES):
        m = dict(common)
        m["hs"] = np.ascontiguousarray(
            hs_np[:, c * TC:(c + 1) * TC, :].reshape(BT, E))
        in_maps.append(m)
    return in_maps


def _get_runner():
    """Build (once) a reusable jitted SPMD callable.

    Weights are replicated across the 8 cores (in_specs=P()); only hs and the
    output are sharded over the leading axis. This avoids the 8x concat +
    retrace of run_bass_kernel_spmd on every call.
    """
    if "runner" in _CACHE:
        return _CACHE["runner"]
    import jax
    from jax.sharding import Mesh, PartitionSpec as P
    from jax.experimental.shard_map import shard_map
    from concourse import bass2jax
    import concourse.mybir as mybir_

    nc = _CACHE.get("nc")
    if nc is None:
        nc = _CACHE["nc"] = build_nc()
    bass2jax.install_neuronx_cc_hook()

    pname = nc.partition_id_tensor.name if nc.partition_id_tensor else None
    in_names, out_names, out_avals = [], [], []
    for alloc in nc.m.functions[0].allocations:
        if not isinstance(alloc, mybir_.MemoryLocationSet):
            continue
        name = alloc.memorylocations[0].name
        if alloc.kind == "ExternalInput":
            if name != pname:
                in_names.append(name)
        elif alloc.kind == "ExternalOutput":
            out_names.append(name)
            shape = tuple(alloc.tensor_shape)
            out_avals.append(jax.core.ShapedArray(shape, mybir_.dt.np(alloc.dtype)))
    n_params = len(in_names)
    all_names = in_names + out_names
    if pname is not None:
        all_names = all_names + [pname]

    def _body(*args):
        operands = list(args)
        if pname is not None:
            operands.append(bass2jax.partition_id_tensor())
        outs = bass2jax._bass_exec_p.bind(
            *operands,
            out_avals=tuple(out_avals),
            in_names=tuple(all_names),
            out_names=tuple(out_names),
            lowering_input_output_aliases=(),
            sim_require_finite=True,
            sim_require_nnan=True,
            nc=nc,
        )
        return tuple(outs)

    devices = jax.devices()[:NCORES]
    mesh = Mesh(np.asarray(devices), ("core",))
    in_specs = tuple(P("core") if n == "hs" else P() for n in in_names)
    in_specs = in_specs + (P("core"),) * len(out_names)
    out_specs = (P("core"),) * len(out_names)
    fn = jax.jit(shard_map(_body, mesh=mesh, in_specs=in_specs,
                           out_specs=out_specs, check_rep=False))

    def _chain(n):
        def body_n(*args):
            ins, outbuf = args[:n_params], args[n_params]
            for _ in range(n):
                (outbuf,) = _body(*ins, outbuf)
            return (outbuf,)
        return jax.jit(shard_map(body_n, mesh=mesh, in_specs=in_specs,
                                 out_specs=out_specs, check_rep=False))

    runner = (fn, in_names, out_names, out_avals, mesh, _chain)
    _CACHE["runner"] = runner
    return runner


def _device_args(in_maps):
    """Assemble the jit arguments (host-side) for the runner."""
    fn, in_names, out_names, out_avals, mesh, _chain = _get_runner()
    args = []
    for n in in_names:
        if n == "hs":
            args.append(np.concatenate([m["hs"] for m in in_maps], axis=0))
        else:
            args.append(in_maps[0][n])
    for av in out_avals:
        args.append(np.zeros((NCORES * av.shape[0],) + av.shape[1:], av.dtype))
    return args


def kernel(**inputs) -> np.ndarray:
    fn, in_names, out_names, out_avals, mesh, _chain = _get_runner()
    in_maps = _prep_inputs(**inputs)
    args = _device_args(in_maps)
    outs = fn(*args)
    out = np.asarray(outs[0])  # (8*4096, 2048)
    return out.reshape(NCORES, B, TC, U, O).transpose(1, 0, 2, 3, 4).reshape(B, T, U, O)


if __name__ == "__main__":
    import time
    t0 = time.time()
    nc = build_nc(steps=int(sys.argv[1]) if len(sys.argv) > 1 else U)
    print(f"built ok in {time.time()-t0:.1f}s", flush=True)



# revision 3
# speedup vs baseline: 1.7243x; 1.7243x over previous
"""RNN-T decoder (embedding + 2-layer LSTM + joint network) on 8 Trainium2 cores.

v2 strategy (fp8 DoubleRow recurrence):
  - LSTM runs replicated on all 8 cores. The recurrent matmul streams W_hh as
    fp8(e4m3, x64) in DoubleRow perf mode (256 contraction rows per pass, 0.5
    cycles/output-row): 4x less PE time than the fp32r baseline. h is
    quantized to fp8 (x64) each step; the batched x-projection gx is
    quantized to fp8 (x32) and injected into PSUM via a DoubleRow matmul
    against a constant "double identity" whose entries are 128, so all PSUM
    contributions carry the same 4096x scale. Activations then apply
    tanh(ps/8192) (== sigmoid pre-halving) or tanh(ps/4096) for the g gate.
  - The elementwise c/h chain runs in a TRANSPOSED [128, 32] layout (gate
    values are transposed by the PE right after activation), which cuts
    DVE/ACT cost ~4x vs the [4, 512] layout and directly produces h^T tiles
    for the next step's matmul (fp8) and for xproj/joint (bf16).
  - Joint network is sharded over T (16 cols/core); zt/W_dec/W_out in bf16.

kernel(**inputs) takes the full unsharded inputs (as in reference.setup_inputs)
and returns the full (B, T, U, ODIM) float32 output.
"""
import sys
import numpy as np

sys.path.insert(0, "/opt/trn_rl_repo")

import concourse.bass as bass
import concourse.bacc as bacc
import concourse.mybir as mybir
import concourse.tile as tile
from contextlib import ExitStack

F32 = mybir.dt.float32
F32R = mybir.dt.float32r
BF16 = mybir.dt.bfloat16
F8 = mybir.dt.float8e4
I32 = mybir.dt.int32
AF = mybir.ActivationFunctionType
OP = mybir.AluOpType
DR = mybir.MatmulPerfMode.DoubleRow

B, T, U = 4, 128, 64
NCORES = 8
TC = T // NCORES          # 16 T-columns per core
E, D, G = 512, 1024, 4096  # embed, dunits, 4*dunits
J, O = 512, 2048           # joint dim, odim
UB = U * B                 # 256, u-major token index (col = 4u+b)
BT = B * TC                # 64 encoder rows per core
KD = D // 128              # 8 contraction chunks of hidden dim
DC = D // 256              # 4 double-chunks for DoubleRow

_CACHE = {}


def _mm_r(nc, out, lhsT, rhs, **kw):
    """matmul with both operands viewed as float32r (full-rate fp32 storage)."""
    nc.tensor.matmul(out, lhsT=lhsT.bitcast(F32R), rhs=rhs.bitcast(F32R), **kw)


def _emit_xproj_q(nc, pools, rhs_of_ec, w_of_ec, nchunks, bih, bhh, gx8,
                  lhsT_bitcast=None):
    """gx8[2*nb+cp][:, i*256:(i+1)*256] = fp8(16 * ((W @ x^T) + bias)).

    rhs_of_ec: ec -> [128, 256] rhs tile (x^T chunk, K on partitions)
    w_of_ec:   ec -> [128, G] tile of W.T rows for that chunk (lhsT source)
    """
    pbig, bpool = pools["pbig"], pools["bias"]
    ones_r = pools["ones"]
    for gm in range(32):
        nb, c = gm // 4, gm % 4
        gs = slice(128 * c, 128 * c + 128)
        if c == 0:
            cb = slice(nb * 512, nb * 512 + 512)
            ba = bpool.tile([1, 512], F32R, tag="ba", name="ba")
            bb = bpool.tile([1, 512], F32, tag="bb", name="bb")
            nc.sync.dma_start(ba[:1, :], bih[None, cb].bitcast(F32R))
            nc.sync.dma_start(bb[:1, :], bhh[None, cb])
            nc.vector.tensor_tensor(ba[:1, :], in0=ba[:1, :],
                                    in1=bb[:1, :], op=OP.add)
        ps = pbig.tile([128, 256], F32, tag="pbig", name="pbig")
        for ec in range(nchunks):
            w = w_of_ec(ec)[:, 128 * gm: 128 * (gm + 1)]
            r = rhs_of_ec(ec)
            if lhsT_bitcast is None:
                _mm_r(nc, ps[:], lhsT=w, rhs=r,
                      start=(ec == 0), stop=False)
            else:
                nc.tensor.matmul(ps[:], lhsT=w, rhs=r,
                                 start=(ec == 0), stop=False)
        _mm_r(nc, ps[:], lhsT=ba[:1, gs], rhs=ones_r[:1, :256],
              start=False, stop=True)
        # quantize to fp8 with x16 scale into the paired layout
        nc.vector.tensor_scalar(
            gx8[2 * nb + c // 2][:, (c % 2) * 256:(c % 2) * 256 + 256],
            ps[:], 32.0, None, OP.mult)


def _emit_lstm_fp8(nc, pools, whh8, gx8, hT16, cT, h8ab, steps, heat=0):
    """One LSTM layer in fp8 DoubleRow, `steps` sequential steps.

    whh8: 4 SBUF tiles [128, 2*G] fp8 (64x W_hh.T), dc-major double-chunks
    gx8: 16 SBUF tiles [128, 512] fp8 (16x (W_ih x + b)), paired layout
    hT16: [128, 32*(U+1)] bf16; col block u+1 <- h[u]^T, cols (i,dc,b)-major
    cT: [128, 32] f32 persistent (caller memsets)
    h8ab: two [128, 32] fp8 tiles, ping-pong (caller memsets h8ab[0])
    """
    pg8, ptT = pools["pgates"], pools["ptT"]
    tpool = pools["tsb"]
    ident8 = pools["ident8"]
    identb = pools["identb"]
    chain = pools["chain"]
    tc_t = pools["tc_t"]
    # processing order: i, g, f, o  (gate index 0,2,1,3) so the post-o tail
    # is only so/h8/h16
    ORDER = (0, 2, 1, 3)
    for u in range(steps):
        h8prev = h8ab[u % 2]
        h8cur = h8ab[(u + 1) % 2]
        lhs_h = h8prev[:].rearrange("p (i dc b) -> p dc i b", i=2, dc=DC, b=4)
        tT = ptT.tile([128, 128], BF16, tag="tT", name="tT")
        tsb = {}
        # --- all matmuls first (PE queue order) ---
        pgs = {}
        for gt in ORDER:
            pg = pg8.tile([4, 1024], F32, tag="pg", name="pg")
            pgs[gt] = pg
            for nh in range(2):
                nb = 2 * gt + nh
                out = pg[:, 512 * nh: 512 * nh + 512]
                for dc in range(DC):
                    rhs = whh8[dc][:].rearrange("p (i n) -> p i n", i=2)[
                        :, :, 512 * nb: 512 * nb + 512]
                    nc.tensor.matmul(out, lhsT=lhs_h[:, dc], rhs=rhs,
                                     start=(dc == 0), stop=False,
                                     perf_mode=DR)
                for cp in range(2):
                    lg = gx8[2 * nb + cp][:].rearrange(
                        "p (i ub) -> p i ub", i=2)[:, :, 4 * u: 4 * u + 4]
                    nc.tensor.matmul(
                        pg[:, 512 * nh + 256 * cp: 512 * nh + 256 * cp + 256],
                        lhsT=lg, rhs=ident8[:].rearrange(
                            "p (i n) -> p i n", i=2),
                        start=False, stop=(cp == 1), perf_mode=DR)
        # --- activations + transposes + chain, in processing order ---
        for oi, gt in enumerate(ORDER):
            scale = (1.0 / 4096.0) if gt == 2 else (1.0 / 8192.0)
            t_sb = tpool.tile([4, 1024], BF16, tag="tsb", name="tsb")
            tsb[gt] = t_sb
            nc.scalar.activation(t_sb[:], pgs[gt][:], AF.Tanh, scale=scale)
            for k in range(8):
                ck = 16 * (k % 2) + 4 * (k // 2)
                nc.tensor.transpose(
                    tT[:, 32 * gt + ck: 32 * gt + ck + 4],
                    in_=t_sb[:, 128 * k: 128 * k + 128],
                    identity=identb[:4, :4])
            ts = tT[:, 32 * gt: 32 * gt + 32]
            if gt == 0:      # i
                si = chain.tile([128, 32], F32, tag="si", name="si")
                pools["si"] = si
                nc.vector.tensor_scalar(si[:], ts, 0.5, 0.5, OP.mult, OP.add)
            elif gt == 2:    # g
                ig = chain.tile([128, 32], F32, tag="ig", name="ig")
                pools["ig"] = ig
                nc.vector.tensor_tensor(ig[:], in0=pools["si"][:], in1=ts,
                                        op=OP.mult)
            elif gt == 1:    # f
                sf = chain.tile([128, 32], F32, tag="sf", name="sf")
                nc.vector.tensor_scalar(sf[:], ts, 0.5, 0.5, OP.mult, OP.add)
                nc.vector.tensor_tensor(sf[:], in0=sf[:], in1=cT[:], op=OP.mult)
                nc.vector.tensor_tensor(cT[:], in0=sf[:], in1=pools["ig"][:],
                                        op=OP.add)
                nc.scalar.activation(tc_t[:], cT[:], AF.Tanh)
            else:            # o
                so = chain.tile([128, 32], F32, tag="so", name="so")
                nc.vector.tensor_scalar(so[:], ts, 32.0, 32.0, OP.mult, OP.add)
                nc.vector.tensor_tensor(h8cur[:], in0=so[:], in1=tc_t[:],
                                        op=OP.mult)
                nc.vector.tensor_scalar(
                    hT16[:, 32 * (u + 1): 32 * (u + 2)], h8cur[:],
                    1.0 / 64.0, None, OP.mult)
        # optional PE heater: keep the tensor engine clocked during the tail
        for _ in range(heat):
            hp = pools["pheat"].tile([4, 512], F32, tag="ph", name="ph")
            nc.tensor.matmul(hp[:], lhsT=lhs_h[:, 0],
                             rhs=whh8[0][:].rearrange(
                                 "p (i n) -> p i n", i=2)[:, :, :512],
                             start=True, stop=True, perf_mode=DR)


def build_nc(steps=U, layers=2, joint=True, heat=0):
    nc = bacc.Bacc("TRN2", target_bir_lowering=False, debug=False)

    hs = nc.dram_tensor("hs", [BT, E], F32, kind="ExternalInput")
    ys_idx = nc.dram_tensor("ys_idx", [UB], I32, kind="ExternalInput")
    embed = nc.dram_tensor("embed", [O, E], F32, kind="ExternalInput")
    wih0T = nc.dram_tensor("wih0T", [E, G], F32R, kind="ExternalInput")
    wih1T16 = nc.dram_tensor("wih1T16", [D, G], BF16, kind="ExternalInput")
    whh8_0 = nc.dram_tensor("whh8_0", [4 * 128, 2 * G], F8, kind="ExternalInput")
    whh8_1 = nc.dram_tensor("whh8_1", [4 * 128, 2 * G], F8, kind="ExternalInput")
    bih0 = nc.dram_tensor("bih0", [G], F32, kind="ExternalInput")
    bhh0 = nc.dram_tensor("bhh0", [G], F32, kind="ExternalInput")
    bih1 = nc.dram_tensor("bih1", [G], F32, kind="ExternalInput")
    bhh1 = nc.dram_tensor("bhh1", [G], F32, kind="ExternalInput")
    wencT = nc.dram_tensor("wencT", [E, J], F32R, kind="ExternalInput")
    benc = nc.dram_tensor("benc", [J], F32R, kind="ExternalInput")
    wdecT16 = nc.dram_tensor("wdecT16", [D, J], BF16, kind="ExternalInput")
    woutT16 = nc.dram_tensor("woutT16", [J, O], BF16, kind="ExternalInput")
    bout_bc = nc.dram_tensor("bout_bc", [128, O], F32, kind="ExternalInput")
    ones_d = nc.dram_tensor("ones_d", [256], F32R, kind="ExternalInput")
    ident_f = nc.dram_tensor("ident_f", [128, 128], F32, kind="ExternalInput")
    ident_b = nc.dram_tensor("ident_b", [128, 128], BF16, kind="ExternalInput")
    ident_8 = nc.dram_tensor("ident_8", [128, 512], F8, kind="ExternalInput")
    out = nc.dram_tensor("out", [BT * U, O], F32, kind="ExternalOutput")

    with tile.TileContext(nc) as tc, ExitStack() as es:
        cpool = es.enter_context(tc.tile_pool(name="const", bufs=1))
        ppool = es.enter_context(tc.tile_pool(name="persist", bufs=1))

        ident = cpool.tile([128, 128], F32, tag="ident", name="ident")
        nc.sync.dma_start(ident[:], ident_f[:])
        ones_r = cpool.tile([1, 256], F32R, tag="ones", name="ones")
        nc.sync.dma_start(ones_r[:1, :], ones_d[None, :])
        identb = cpool.tile([128, 128], BF16, tag="identb", name="identb")
        nc.sync.dma_start(identb[:], ident_b[:])
        ident8 = cpool.tile([128, 512], F8, tag="ident8", name="ident8")
        nc.sync.dma_start(ident8[:], ident_8[:])

        gx8 = [ppool.tile([128, 512], F8, tag=f"gx8_{g}", name=f"gx8_{g}")
               for g in range(16)]
        hT16_0 = ppool.tile([128, 32 * (U + 1)], BF16, tag="hT16_0", name="hT16_0")
        hT16_1 = ppool.tile([128, 32 * (U + 1)], BF16, tag="hT16_1", name="hT16_1")
        cT = ppool.tile([128, 32], F32, tag="cT", name="cT")
        h8a = ppool.tile([128, 32], F8, tag="h8a", name="h8a")
        h8b = ppool.tile([128, 32], F8, tag="h8b", name="h8b")
        tc_t = ppool.tile([128, 32], BF16, tag="tc_t", name="tc_t")
        whh0 = [ppool.tile([128, 2 * G], F8, tag=f"whh0_{d}", name=f"whh0_{d}")
                for d in range(DC)]
        whh1 = [ppool.tile([128, 2 * G], F8, tag=f"whh1_{d}", name=f"whh1_{d}")
                for d in range(DC)]

        pools = {"ones": ones_r, "identb": identb, "ident8": ident8,
                 "tc_t": tc_t}

        wenc = [ppool.tile([128, J], F32R, tag=f"wenc{ec}", name=f"wenc{ec}")
                for ec in range(4)]
        benc_sb = ppool.tile([1, J], F32R, tag="benc", name="benc")
        hs_sb = ppool.tile([BT, E], F32, tag="hs_sb", name="hs_sb")
        hsT = [ppool.tile([128, BT], F32R, tag=f"hsT{ec}", name=f"hsT{ec}")
               for ec in range(4)]
        encp = [ppool.tile([128, BT], F32, tag=f"encp{jt}", name=f"encp{jt}")
                for jt in range(4)]

        # ---- Phase 1+2: embedding gather -> eys^T, layer-0 x-projection ----
        with tc.tile_pool(name="ph2", bufs=1) as p2, \
             tc.tile_pool(name="bias2", bufs=2) as bpool2, \
             tc.tile_pool(name="pbig", bufs=4, space="PSUM") as pbig, \
             tc.tile_pool(name="pT", bufs=2, space="PSUM") as pT:
            pools["pbig"] = pbig
            pools["bias"] = bpool2
            eysT = [p2.tile([128, 256], F32R, tag=f"eysT{ec}", name=f"eysT{ec}")
                    for ec in range(4)]
            idxs = []
            for t in range(2):
                idx = p2.tile([128, 1], I32, tag=f"idx{t}", name=f"idx{t}")
                nc.sync.dma_start(idx[:, :1], ys_idx[128 * t:128 * (t + 1), None])
                idxs.append(idx)
            wih0 = [p2.tile([128, G], F32R, tag=f"wih0_{ec}", name=f"wih0_{ec}")
                    for ec in range(4)]
            for ec in range(4):
                nc.sync.dma_start(wih0[ec][:], wih0T[128 * ec:128 * (ec + 1), :])
            # recurrent weights (both layers) early: overlaps with xproj
            for d in range(DC):
                nc.sync.dma_start(whh0[d][:], whh8_0[128 * d:128 * (d + 1), :])
            for t in range(2):
                idx = idxs[t]
                ey = p2.tile([128, E], F32, tag=f"ey{t}", name=f"ey{t}")
                nc.gpsimd.indirect_dma_start(
                    out=ey[:], out_offset=None, in_=embed[:],
                    in_offset=bass.IndirectOffsetOnAxis(ap=idx[:, :1], axis=0))
                for ec in range(4):
                    tp = pT.tile([128, 128], F32, tag="pT", name="pT")
                    nc.tensor.transpose(tp[:], in_=ey[:, 128 * ec:128 * (ec + 1)],
                                        identity=ident[:])
                    nc.vector.tensor_copy(eysT[ec][:, 128 * t:128 * (t + 1)], tp[:])
            _emit_xproj_q(nc, pools, lambda ec: eysT[ec][:],
                          lambda ec: wih0[ec], 4, bih0, bhh0, gx8)

            for d in range(DC):
                nc.sync.dma_start(whh1[d][:], whh8_1[128 * d:128 * (d + 1), :])
            # encoder-side joint work: hs^T and enc_p^T (PE has slack here)
            for ec in range(4):
                nc.sync.dma_start(wenc[ec][:], wencT[128 * ec:128 * (ec + 1), :])
            nc.sync.dma_start(hs_sb[:], hs[:])
            nc.sync.dma_start(benc_sb[:1, :], benc[None, :])
            for ec in range(4):
                tp = pT.tile([128, 128], F32, tag="pT", name="pT")
                nc.tensor.transpose(tp[:, :BT],
                                    in_=hs_sb[:, 128 * ec:128 * (ec + 1)],
                                    identity=ident[:BT, :BT])
                nc.vector.tensor_copy(hsT[ec][:], tp[:, :BT])
            for jt in range(4):
                tpp = pT.tile([128, 128], F32, tag="pT", name="pT")
                pse = tpp[:, :BT]
                for ec in range(4):
                    _mm_r(nc, pse, lhsT=wenc[ec][:, 128 * jt:128 * (jt + 1)],
                          rhs=hsT[ec][:], start=(ec == 0), stop=False)
                _mm_r(nc, pse, lhsT=benc_sb[:1, 128 * jt:128 * (jt + 1)],
                      rhs=ones_r[:1, :BT], start=False, stop=True)
                nc.vector.tensor_copy(encp[jt][:], pse)

        # ---- Phase 3: layer-0 recurrence ----
        if steps < U:  # debug builds: phases 4/6 read all U step blocks
            nc.gpsimd.memset(hT16_0[:], 0.0)
            nc.gpsimd.memset(hT16_1[:], 0.0)
        nc.gpsimd.memset(cT[:], 0.0)
        nc.gpsimd.memset(h8a[:].bitcast(mybir.dt.uint8), 0)
        with tc.tile_pool(name="pgA", bufs=2, space="PSUM") as pgates, \
             tc.tile_pool(name="ptTA", bufs=2, space="PSUM") as ptT, \
             tc.tile_pool(name="pheatA", bufs=1, space="PSUM") as pheat, \
             tc.tile_pool(name="tsbA", bufs=3) as tsb, \
             tc.tile_pool(name="chainA", bufs=2) as chain:
            pools.update(pgates=pgates, ptT=ptT, tsb=tsb, chain=chain,
                         pheat=pheat)
            _emit_lstm_fp8(nc, pools, whh0, gx8, hT16_0, cT, (h8a, h8b),
                           steps, heat=heat)

        # ---- Phase 4: layer-1 x-projection (streams W_ih1 bf16) ----
        with tc.tile_pool(name="ph4", bufs=1) as p4, \
             tc.tile_pool(name="bias4", bufs=2) as bpool4, \
             tc.tile_pool(name="pbig2", bufs=4, space="PSUM") as pbig2:
            pools["pbig"] = pbig2
            pools["bias"] = bpool4
            wih1 = [p4.tile([128, G], BF16, tag=f"wih1_{k}", name=f"wih1_{k}")
                    for k in range(KD)]
            for k in range(KD):
                nc.sync.dma_start(wih1[k][:], wih1T16[128 * k:128 * (k + 1), :])
            rhs_of = lambda ec: hT16_0[:].rearrange(
                "p (u c b) -> p c u b", u=U + 1, c=8, b=4)[
                :, 4 * (ec % 2) + ec // 2, 1:U + 1]
            _emit_xproj_q(nc, pools, rhs_of, lambda ec: wih1[ec], KD,
                          bih1, bhh1, gx8, lhsT_bitcast=True)

        # ---- Phase 5: layer-1 recurrence ----
        nc.gpsimd.memset(cT[:], 0.0)
        nc.gpsimd.memset(h8a[:].bitcast(mybir.dt.uint8), 0)
        if layers >= 2:
            with tc.tile_pool(name="pgB", bufs=2, space="PSUM") as pgates2, \
                 tc.tile_pool(name="ptTB", bufs=2, space="PSUM") as ptT2, \
                 tc.tile_pool(name="pheatB", bufs=1, space="PSUM") as pheat2, \
                 tc.tile_pool(name="tsbB", bufs=3) as tsb2, \
                 tc.tile_pool(name="chainB", bufs=2) as chain2:
                pools.update(pgates=pgates2, ptT=ptT2, tsb=tsb2, chain=chain2,
                             pheat=pheat2)
                _emit_lstm_fp8(nc, pools, whh1, gx8, hT16_1, cT, (h8a, h8b),
                               steps, heat=heat)
        else:
            nc.vector.tensor_copy(hT16_1[:, 32:32 * (steps + 1)],
                                  hT16_0[:, 32:32 * (steps + 1)])

        # ---- Phase 6: joint network on this core's T-slice ----
        if joint:
            with tc.tile_pool(name="joint", bufs=1) as jp, \
                 tc.tile_pool(name="zt", bufs=4) as ztp, \
                 tc.tile_pool(name="osb", bufs=4) as osbp, \
                 tc.tile_pool(name="pj", bufs=4, space="PSUM") as pj:
                wdec = [jp.tile([128, J], BF16, tag=f"wdec{k}", name=f"wdec{k}")
                        for k in range(KD)]
                wout = [jp.tile([128, O], BF16, tag=f"wout{jt}", name=f"wout{jt}")
                        for jt in range(4)]
                bout_sb = jp.tile([128, O], F32, tag="bout", name="bout")
                decp = [jp.tile([128, 256], F32, tag=f"decp{jt}", name=f"decp{jt}")
                        for jt in range(4)]
                for k in range(KD):
                    nc.sync.dma_start(wdec[k][:], wdecT16[128 * k:128 * (k + 1), :])
                nc.sync.dma_start(bout_sb[:], bout_bc[:])
                for jt in range(4):
                    nc.sync.dma_start(wout[jt][:], woutT16[128 * jt:128 * (jt + 1), :])

                # dec_p^T[jt], columns reordered (b, u)
                for jt in range(4):
                    ps = pj.tile([128, 256], F32, tag="pj", name="pj")
                    for k in range(KD):
                        rhs = hT16_1[:].rearrange(
                            "p (u c b) -> p c b u", u=U + 1, c=8, b=4)[
                            :, 4 * (k % 2) + k // 2, :, 1:U + 1]
                        nc.tensor.matmul(ps[:], lhsT=wdec[k][:, 128 * jt:128 * (jt + 1)],
                                         rhs=rhs, start=(k == 0), stop=(k == KD - 1))
                    nc.vector.tensor_copy(decp[jt][:], ps[:])
                # z^T tiles + output matmul, one M-tile (=2 encoder rows) at a time
                for m in range(BT * U // 128):
                    zt = [ztp.tile([128, 128], BF16, tag=f"zt{jt}", name=f"zt{jt}")
                          for jt in range(4)]
                    for jt in range(4):
                        for half in range(2):
                            bt = 2 * m + half
                            b = bt // TC
                            nc.scalar.activation(
                                zt[jt][:, half * 64:(half + 1) * 64],
                                decp[jt][:, b * 64:(b + 1) * 64],
                                AF.Tanh, bias=encp[jt][:, bt:bt + 1])
                    for ob in range(4):
                        obs = slice(ob * 512, (ob + 1) * 512)
                        ps = pj.tile([128, 512], F32, tag="pj", name="pj")
                        for jt in range(4):
                            nc.tensor.matmul(ps[:], lhsT=zt[jt][:],
                                             rhs=wout[jt][:, obs],
                                             start=(jt == 0), stop=(jt == 3))
                        o_sb = osbp.tile([128, 512], F32, tag="osb", name="osb")
                        nc.vector.tensor_tensor(o_sb[:], in0=ps[:],
                                                in1=bout_sb[:, obs], op=OP.add)
                        nc.sync.dma_start(out[128 * m:128 * (m + 1), obs], o_sb[:])
        else:
            zsrc = ppool.tile([128, 512], F32, tag="zsrc", name="zsrc")
            nc.gpsimd.memset(zsrc[:], 0.0)
            for m0 in range(BT * U // 128):
                for ob in range(4):
                    nc.sync.dma_start(out[128 * m0:128 * (m0 + 1),
                                          ob * 512:(ob + 1) * 512], zsrc[:])

    nc.compile()
    return nc


def _prep_inputs(hs_pad, ys_in_pad, embed, W_ih0, W_hh0, b_ih0, b_hh0,
                 W_ih1, W_hh1, b_ih1, b_hh1, W_enc, b_enc, W_dec, W_out, b_out):
    import ml_dtypes
    E4 = ml_dtypes.float8_e4m3
    BF = ml_dtypes.bfloat16
    f = np.float32
    tr = lambda a: np.ascontiguousarray(np.asarray(a).T, dtype=f)

    def whh8(W):
        WT64 = (np.asarray(W, f).T * 64.0).astype(E4)          # [D, G]
        return np.ascontiguousarray(
            WT64.reshape(DC, 2, 128, G).transpose(0, 2, 1, 3).reshape(4 * 128, 2 * G))

    id8 = np.zeros((128, 512), E4)
    for p in range(128):
        id8[p, p] = 128.0
        id8[p, 256 + 128 + p] = 128.0

    common = {
        "ys_idx": np.ascontiguousarray(np.asarray(ys_in_pad).T.reshape(-1),
                                       dtype=np.int32),
        "embed": np.ascontiguousarray(embed, dtype=f),
        "wih0T": tr(W_ih0),
        "wih1T16": np.ascontiguousarray(np.asarray(W_ih1, f).T.astype(BF)),
        "whh8_0": whh8(W_hh0), "whh8_1": whh8(W_hh1),
        "bih0": np.asarray(b_ih0, f), "bhh0": np.asarray(b_hh0, f),
        "bih1": np.asarray(b_ih1, f), "bhh1": np.asarray(b_hh1, f),
        "wencT": tr(W_enc),
        "wdecT16": np.ascontiguousarray(np.asarray(W_dec, f).T.astype(BF)),
        "woutT16": np.ascontiguousarray(np.asarray(W_out, f).T.astype(BF)),
        "benc": np.asarray(b_enc, f),
        "bout_bc": np.ascontiguousarray(
            np.broadcast_to(np.asarray(b_out, f)[None, :], (128, O))),
        "ones_d": np.ones(256, f),
        "ident_f": np.eye(128, dtype=f),
        "ident_b": np.eye(128).astype(BF),
        "ident_8": id8,
    }
    hs_np = np.asarray(hs_pad, f)
    in_maps = []
    for c in range(NCORES):
        m = dict(common)
        m["hs"] = np.ascontiguousarray(
            hs_np[:, c * TC:(c + 1) * TC, :].reshape(BT, E))
        in_maps.append(m)
    return in_maps



def _get_runner():
    """Build (once) a reusable jitted SPMD callable.

    Weights are replicated across the 8 cores (in_specs=P()); only hs and the
    output are sharded over the leading axis. This avoids the 8x concat +
    retrace of run_bass_kernel_spmd on every call.
    """
    if "runner" in _CACHE:
        return _CACHE["runner"]
    import jax
    from jax.sharding import Mesh, PartitionSpec as P
    from jax.experimental.shard_map import shard_map
    from concourse import bass2jax
    import concourse.mybir as mybir_

    nc = _CACHE.get("nc")
    if nc is None:
        nc = _CACHE["nc"] = build_nc()
    bass2jax.install_neuronx_cc_hook()

    pname = nc.partition_id_tensor.name if nc.partition_id_tensor else None
    in_names, out_names, out_avals = [], [], []
    for alloc in nc.m.functions[0].allocations:
        if not isinstance(alloc, mybir_.MemoryLocationSet):
            continue
        name = alloc.memorylocations[0].name
        if alloc.kind == "ExternalInput":
            if name != pname:
                in_names.append(name)
        elif alloc.kind == "ExternalOutput":
            out_names.append(name)
            shape = tuple(alloc.tensor_shape)
            out_avals.append(jax.core.ShapedArray(shape, mybir_.dt.np(alloc.dtype)))
    n_params = len(in_names)
    all_names = in_names + out_names
    if pname is not None:
        all_names = all_names + [pname]

    def _body(*args):
        operands = list(args)
        if pname is not None:
            operands.append(bass2jax.partition_id_tensor())
        outs = bass2jax._bass_exec_p.bind(
            *operands,
            out_avals=tuple(out_avals),
            in_names=tuple(all_names),
            out_names=tuple(out_names),
            lowering_input_output_aliases=(),
            sim_require_finite=True,
            sim_require_nnan=True,
            nc=nc,
        )
        return tuple(outs)

    devices = jax.devices()[:NCORES]
    mesh = Mesh(np.asarray(devices), ("core",))
    in_specs = tuple(P("core") if n == "hs" else P() for n in in_names)
    in_specs = in_specs + (P("core"),) * len(out_names)
    out_specs = (P("core"),) * len(out_names)
    fn = jax.jit(shard_map(_body, mesh=mesh, in_specs=in_specs,
                           out_specs=out_specs, check_rep=False))

    def _chain(n):
        def body_n(*args):
            ins, outbuf = args[:n_params], args[n_params]
            for _ in range(n):
                (outbuf,) = _body(*ins, outbuf)
            return (outbuf,)
        return jax.jit(shard_map(body_n, mesh=mesh, in_specs=in_specs,
                                 out_specs=out_specs, check_rep=False))

    runner = (fn, in_names, out_names, out_avals, mesh, _chain)
    _CACHE["runner"] = runner
    return runner


def _device_args(in_maps):
    """Assemble the jit arguments (host-side) for the runner."""
    fn, in_names, out_names, out_avals, mesh, _chain = _get_runner()
    args = []
    for n in in_names:
        if n == "hs":
            args.append(np.concatenate([m["hs"] for m in in_maps], axis=0))
        else:
            args.append(in_maps[0][n])
    for av in out_avals:
        args.append(np.zeros((NCORES * av.shape[0],) + av.shape[1:], av.dtype))
    return args


def kernel(**inputs) -> np.ndarray:
    fn, in_names, out_names, out_avals, mesh, _chain = _get_runner()
    in_maps = _prep_inputs(**inputs)
    args = _device_args(in_maps)
    outs = fn(*args)
    out = np.asarray(outs[0])  # (8*4096, 2048)
    return out.reshape(NCORES, B, TC, U, O).transpose(1, 0, 2, 3, 4).reshape(B, T, U, O)


if __name__ == "__main__":
    import time
    t0 = time.time()
    nc = build_nc(steps=int(sys.argv[1]) if len(sys.argv) > 1 else U)
    print(f"built ok in {time.time()-t0:.1f}s", flush=True)

